# revision 41
# baseline (speedup 1.0000x reference)
"""Trainium2 Bass kernel for nn_MHAAttention (LayerNorm2d + MHA w/ rel-pos bias + residual).

Sharding: data-parallel over batch - 8 batch elements, one per NeuronCore.

v2 design (vs the 183us baseline):
  - Head PAIRS interleaved: heads (2p, 2p+1) live at partitions 0-63 / 64-127
    of the same qT/kT tile, so their K=64 score matmuls go to different PE
    row-groups (tile_position auto) and run CONCURRENTLY (~2x score phase).
  - attn@V lag-2 carries across the pair boundary: next pair's scores are
    emitted before this pair's tail attn@V, so PE never drains (kills the
    per-head ~2.6us bubble + HAM re-throttle of the baseline).
  - v layout per head is [1 | v(64)]: the ones FIRST, so the softmax
    denominator Z lands at o partition 0 and 1/Z is computed with
    nc.vector.reciprocal_approx_fast on the (1, N) row (DVE), replicated by
    nc.gpsimd.partition_broadcast - the ScalarE ln/exp z-path is GONE;
    ScalarE does (almost) nothing but the 64 mandatory score-exp evacs.
  - Projection uses K=128 pair-stacked matmuls: wp2[0:64]=even head,
    wp2[64:128]=odd head rows; oTn2 stacked the same way via a small
    SBUF->SBUF DMA partition shift. Halves proj matmul count.
  - bp is folded into the residual x host-side; LN affine + 1/sqrt(d) are
    folded into wq/wk/wv host-side (as baseline).
  - QK for pair p+1 and the V projections are emitted INSIDE the attention
    stream (attention is ACT-bound; PE/DVE have slack), proj for pairs 0-2
    rides inside pair 3.
  - Evacuations balanced across engines: q on ScalarE (Identity + per-
    partition bias AP), k on DVE, v on ScalarE (Copy), all during attention.
"""

import sys

for _p in ("/opt/trn_rl_repo",):
    if _p not in sys.path:
        sys.path.insert(0, _p)

from contextlib import ExitStack

import numpy as np
import ml_dtypes

import concourse.bass as bass
import concourse.mybir as mybir
import concourse.tile as tile
from concourse import library_config
from concourse.bass_utils import run_bass_kernel_spmd

F32 = mybir.dt.float32
BF16 = mybir.dt.bfloat16
AF = mybir.ActivationFunctionType
OP = mybir.AluOpType

B = 8
CH = 512
H = W = 32
NT = H * W          # 1024 tokens
HEADS = 8
HD = 64
EPS = 1e-6
P = 128
CT = CH // P        # 4 channel tiles
TT = NT // P        # 8 token tiles
IC = NT // 512      # 2 free-dim chunks of 512
NPAIR = HEADS // 2  # 4 head pairs
STRIP_W = 60 * 32   # 1920
VW = 66             # per-head v stride: [1 | v(64) | pad]


def _build_strips(rel: np.ndarray) -> np.ndarray:
    """(3969, 8) rel table -> (8, 128, 1920) bias strips (see baseline docstring).

    strip[h, 32*jh_l + jw, 32*g + iw] = T_h[g - jh_l + 3, iw - jw + 31]
    bias.T block for key-tile jt is strip[:, (28-4*jt)*32 : +1024].
    """
    T = rel.reshape(63, 63, HEADS)
    jh_l = np.arange(4)[:, None, None, None]
    jw = np.arange(32)[None, :, None, None]
    g = np.arange(60)[None, None, :, None]
    iw = np.arange(32)[None, None, None, :]
    a = g - jh_l + 3
    b = iw - jw + 31
    a_b, b_b = np.broadcast_arrays(a, b)
    out = T[a_b, b_b, :]
    out = np.ascontiguousarray(np.moveaxis(out, -1, 0)).reshape(HEADS, 128, STRIP_W)
    return out


def _build_nc() -> bass.Bass:
    nc = bass.Bass()

    x_d = nc.declare_dram_parameter("x", [CH, NT], F32, isOutput=False)  # x + bp
    xb_d = nc.declare_dram_parameter("xb", [CH, NT], BF16, isOutput=False)
    wqT_d = nc.declare_dram_parameter("wqT", [CH, CH], BF16, isOutput=False)
    wkT_d = nc.declare_dram_parameter("wkT", [CH, CH], BF16, isOutput=False)
    wvT_d = nc.declare_dram_parameter("wvT", [CH, CH], BF16, isOutput=False)
    wp2_d = nc.declare_dram_parameter("wp2", [P, NPAIR, CH], BF16, isOutput=False)
    # aug rows for the raw-x Q/K of pair 0: [q/k][wsum; bias][d in dtl0]
    aug_d = nc.declare_dram_parameter("aug", [2, 2, P], BF16, isOutput=False)
    bqk_d = nc.declare_dram_parameter("bqk", [2, CH], F32, isOutput=False)
    bv_d = nc.declare_dram_parameter("bv", [1, CH], BF16, isOutput=False)
    strips_d = nc.declare_dram_parameter("strips", [HEADS, P, STRIP_W], BF16,
                                         isOutput=False)
    y_d = nc.declare_dram_parameter("y", [CH, NT], F32, isOutput=True)

    with tile.TileContext(nc) as tc, ExitStack() as ctx:
        singles = ctx.enter_context(tc.tile_pool(name="singles", bufs=1))
        work = ctx.enter_context(tc.tile_pool(name="work", bufs=4))
        es_pool = ctx.enter_context(tc.tile_pool(name="es_pool", bufs=4))
        at_pool = ctx.enter_context(tc.tile_pool(name="at_pool", bufs=6))
        strip_pool = ctx.enter_context(tc.tile_pool(name="strip_pool", bufs=4))
        zw = ctx.enter_context(tc.tile_pool(name="zw", bufs=2))
        # PSUM: big 2x(128,1024)=4 banks + o 2x(65,1024)=4 banks = 8 banks.
        big = ctx.enter_context(tc.tile_pool(name="big", bufs=2, space="PSUM"))
        ps_o = ctx.enter_context(tc.tile_pool(name="ps_o", bufs=2, space="PSUM"))

        # ---------- persistent SBUF ----------
        x_sb = singles.tile([P, CT, NT], F32)        # residual x + bp
        xb_sb = singles.tile([P, CT, NT], BF16)
        xn_sb = singles.tile([P, CT, NT], BF16)
        qT_sb = singles.tile([P, CT, NT], BF16)      # (d part, t free), pair p at ct=p
        kT_sb = singles.tile([P, CT, NT], BF16)
        v_sb = singles.tile([P, TT, HEADS * VW], BF16)
        oTn2_sb = singles.tile([P, NPAIR, NT], BF16)  # pair-stacked normalized oT
        y012_sb = singles.tile([P, CT, NT], F32)     # x + proj(pairs 0-2)

        wq_sb = singles.tile([P, CT, CH], BF16)
        wk_sb = singles.tile([P, CT, CH], BF16)
        wv_sb = singles.tile([P, CT, CH], BF16)
        wp2_sb = singles.tile([P, NPAIR, CH], BF16)
        bqk_sb = singles.tile([P, 2, CT], F32)
        bv_sb = singles.tile([1, CH], BF16)
        aug_sb = singles.tile([2, 2, P], BF16)
        augrow = singles.tile([2, NT], BF16)   # [negmu; invrs] rows
        ones_mb = singles.tile([P, P], BF16)
        ones_rb = singles.tile([1, 512], BF16)
        zeros_mb = singles.tile([P, P], BF16)  # HAM-warmer lhsT

        mu_b = singles.tile([P, NT], BF16)
        rs_b = singles.tile([P, NT], BF16)
        m2_f = singles.tile([P, NT], F32)
        ve_f = singles.tile([P, NT], F32)

        nc.vector.memset(ones_mb[:], 1.0)
        nc.vector.memset(ones_rb[:], 1.0)
        nc.vector.memset(zeros_mb[:], 0.0)

        xb_r = xb_d.rearrange("(ct p) t -> p ct t", p=P)
        # ct0 in two halves so LN stats start ASAP
        nc.sync.dma_start(xb_sb[:, 0, 0:512], xb_r[:, 0, 0:512])
        nc.sync.dma_start(xb_sb[:, 0, 512:1024], xb_r[:, 0, 512:1024])
        for ct in range(1, CT):
            nc.sync.dma_start(xb_sb[:, ct], xb_r[:, ct])
        nc.sync.dma_start(bqk_sb[:], bqk_d.rearrange("i (o p) -> p i o", p=P))
        nc.sync.dma_start(aug_sb[:], aug_d[:])
        nc.sync.dma_start(wq_sb[:], wqT_d.rearrange("(ck p) d -> p ck d", p=P))
        nc.sync.dma_start(wk_sb[:], wkT_d.rearrange("(ck p) d -> p ck d", p=P))
        nc.sync.dma_start(wv_sb[:], wvT_d.rearrange("(ck p) d -> p ck d", p=P))
        nc.sync.dma_start(bv_sb[:], bv_d[:])

        # ones column of v (first col of each head's VW block)
        v_view = v_sb[:].rearrange("p tt (h w) -> p tt h w", w=VW)
        nc.vector.memset(v_view[:, :, :, 0:1], 1.0)

        # ---------- LayerNorm stats ----------
        sum_ps = big.tile([P, NT], F32, tag="big", name="sum_ps")
        sq_ps = big.tile([P, NT], F32, tag="big", name="sq_ps")
        for ct in range(CT):
            x2 = work.tile([P, NT], BF16, name=f"x2_{ct}", tag="x2")
            nc.scalar.activation(out=x2[:], in_=xb_sb[:, ct], func=AF.Square)
            for ic in range(IC):
                sl = slice(ic * 512, ic * 512 + 512)
                nc.tensor.matmul(sum_ps[:, sl], lhsT=ones_mb[:],
                                 rhs=xb_sb[:, ct, sl],
                                 start=(ct == 0), stop=(ct == CT - 1))
                nc.tensor.matmul(sq_ps[:, sl], lhsT=ones_mb[:], rhs=x2[:, sl],
                                 start=(ct == 0), stop=(ct == CT - 1))

        # late-ish DMAs (after the critical first wave)
        x_r = x_d.rearrange("(ct p) t -> p ct t", p=P)
        for ct in range(CT):
            nc.sync.dma_start(x_sb[:, ct], x_r[:, ct])
        strip_tiles = {}
        for h in (0, 1):
            st = strip_pool.tile([P, STRIP_W], BF16, name=f"strip{h}", tag="strip")
            nc.sync.dma_start(st[:], strips_d[h])
            strip_tiles[h] = st
        nc.sync.dma_start(wp2_sb[:], wp2_d[:])

        # ---------- LN scalar chain ----------
        nc.scalar.activation(out=mu_b[:], in_=sum_ps[:], func=AF.Copy,
                             scale=1.0 / CH)
        nc.vector.tensor_tensor(out=m2_f[:], in0=mu_b[:], in1=sum_ps[:],
                                op=OP.mult)
        nc.vector.scalar_tensor_tensor(out=ve_f[:], in0=sq_ps[:],
                                       scalar=float(CH * EPS), in1=m2_f[:],
                                       op0=OP.add, op1=OP.subtract)
        nc.scalar.activation(out=ve_f[:], in_=ve_f[:], func=AF.Ln,
                             scale=1.0 / CH)
        nc.scalar.activation(out=rs_b[:], in_=ve_f[:], func=AF.Exp,
                             scale=-0.5)
        # aug rows for raw-x Q/K: negmu = -sum/CH, invrs = sqrt(var+eps)
        # (engines can only write at partition 0; DMA shifts row 1 into place)
        nc.scalar.activation(out=augrow[0:1, :], in_=sum_ps[0:1, :],
                             func=AF.Copy, scale=-1.0 / CH)
        invrs_tmp = work.tile([1, NT], BF16, name="invrs_tmp", tag="invrs")
        nc.scalar.activation(out=invrs_tmp[:], in_=ve_f[0:1, :],
                             func=AF.Exp, scale=0.5)
        nc.sync.dma_start(augrow[1:2, :], invrs_tmp[:])

        # ---------- QK / V tile emitters ----------
        def apply_ln(ct):
            nc.vector.tensor_tensor(out=xn_sb[:, ct], in0=xb_sb[:, ct],
                                    in1=mu_b[:], op=OP.subtract)
            nc.vector.tensor_tensor(out=xn_sb[:, ct], in0=xn_sb[:, ct],
                                    in1=rs_b[:], op=OP.mult)

        def emit_q1(dtl, wsb, qk, dst):
            """one projection tile (q or k) for head-pair dtl, xn-based"""
            dsl = slice(dtl * P, dtl * P + P)
            ps = big.tile([P, NT], F32, tag="big", name=f"{qk}_ps{dtl}")
            for ck in range(CT):
                for ic in range(IC):
                    sl = slice(ic * 512, ic * 512 + 512)
                    nc.tensor.matmul(ps[:, sl], lhsT=wsb[:, ck, dsl],
                                     rhs=xn_sb[:, ck, sl],
                                     start=(ck == 0), stop=(ck == CT - 1))
            nc.vector.tensor_scalar_add(out=dst[:, dtl], in0=ps[:],
                                        scalar1=bqk_sb[:, qk, dtl : dtl + 1])

        def emit_q(dtl):
            emit_q1(dtl, wq_sb, 0, qT_sb)

        def emit_k(dtl):
            emit_q1(dtl, wk_sb, 1, kT_sb)

        def emit_qk0_raw():
            """Q,K for pair 0 from RAW xb: no LN-chain dependency for the
            matmuls.  q = rs*(wq.T@xb - mu*colsum(wq) + bq*invrs), where the
            last two terms ride a K=2 aug matmul whose rhs rows [negmu;
            invrs] come from the scalar chain (only the 9th matmul waits)."""
            q_ps = big.tile([P, NT], F32, tag="big", name="q_ps0")
            k_ps = big.tile([P, NT], F32, tag="big", name="k_ps0")
            dsl = slice(0, P)
            for ck in range(CT):
                for ic in range(IC):
                    sl = slice(ic * 512, ic * 512 + 512)
                    nc.tensor.matmul(q_ps[:, sl], lhsT=wq_sb[:, ck, dsl],
                                     rhs=xb_sb[:, ck, sl],
                                     start=(ck == 0), stop=False)
                for ic in range(IC):
                    sl = slice(ic * 512, ic * 512 + 512)
                    nc.tensor.matmul(k_ps[:, sl], lhsT=wk_sb[:, ck, dsl],
                                     rhs=xb_sb[:, ck, sl],
                                     start=(ck == 0), stop=False)
            for ic in range(IC):
                sl = slice(ic * 512, ic * 512 + 512)
                nc.tensor.matmul(q_ps[:, sl], lhsT=aug_sb[:, 0, :],
                                 rhs=augrow[:, sl], start=False, stop=True)
                nc.tensor.matmul(k_ps[:, sl], lhsT=aug_sb[:, 1, :],
                                 rhs=augrow[:, sl], start=False, stop=True)
            nc.vector.tensor_tensor(out=qT_sb[:, 0], in0=q_ps[:],
                                    in1=rs_b[:], op=OP.mult)
            nc.vector.tensor_tensor(out=kT_sb[:, 0], in0=k_ps[:],
                                    in1=rs_b[:], op=OP.mult)

        def emit_v(tt2):
            """v for token tiles (2*tt2, 2*tt2+1) in one (128,1024) psum."""
            v_ps = big.tile([P, NT], F32, tag="big", name=f"v_ps{tt2}")
            for j in range(2):
                tt = 2 * tt2 + j
                tsl = slice(tt * P, tt * P + P)
                osl = slice(j * 512, j * 512 + 512)
                for ck in range(CT):
                    nc.tensor.matmul(v_ps[:, osl], lhsT=xn_sb[:, ck, tsl],
                                     rhs=wv_sb[:, ck, :],
                                     start=(ck == 0), stop=False)
                nc.tensor.matmul(v_ps[:, osl], lhsT=ones_rb[:, :P],
                                 rhs=bv_sb[:], start=False, stop=True)
            nc.vector.tensor_copy(
                out=v_view[:, 2 * tt2 : 2 * tt2 + 2, :, 1 : HD + 1],
                in_=v_ps[:].rearrange("p (tt h w) -> p tt h w", tt=2, w=HD))

        emit_qk0_raw()
        for ct in range(CT):
            apply_ln(ct)
        emit_v(0)

        # ---------- attention: pair-interleaved with lag-2 across pairs ----
        # filler schedule: slot (p, jt) -> list of thunks
        fillers = {}
        fillers[(0, 0)] = [lambda: emit_v(1)]
        fillers[(0, 2)] = [lambda: emit_v(2)]
        fillers[(0, 4)] = [lambda: emit_v(3)]
        fillers[(0, 5)] = [lambda: emit_q(1)]
        fillers[(0, 6)] = [lambda: emit_k(1)]
        fillers[(1, 2)] = [lambda: emit_q(2)]
        fillers[(1, 4)] = [lambda: emit_k(2)]
        fillers[(2, 2)] = [lambda: emit_q(3)]
        fillers[(2, 4)] = [lambda: emit_k(3)]

        o_tiles = {}       # head -> psum tile
        at_tiles = {}      # (head, jt) -> aT tile
        pend_av = []       # deferred attn@V work: (head, jt)

        def emit_attnv(h, l):
            o = o_tiles[h]
            aT = at_tiles.pop((h, l))
            for ic in range(IC):
                sl = slice(ic * 512, ic * 512 + 512)
                nc.tensor.matmul(
                    o[:, sl],
                    lhsT=v_sb[:, l, h * VW : h * VW + HD + 1],
                    rhs=aT[:, sl],
                    start=(l == 0), stop=(l == TT - 1))

        pend_z2 = []

        def emit_z1(h, p):
            """z part 1: 1/Z rows on ACT + raw-o evacuation (frees psum).

            o row 0 is Z (ones col is FIRST in v).  1/Z = exp(-ln Z) stays on
            the natural_log_exp ACT table set.
            """
            o = o_tiles.pop(h)
            lnz = zw.tile([1, NT], F32, name=f"lnz{h}", tag="lnz")
            rzb = zw.tile([1, NT], BF16, name=f"rzb{h}", tag="rzb")
            oraw = zw.tile([HD + 1, NT], BF16, name=f"oraw{h}", tag="oraw")
            nc.scalar.activation(out=lnz[:], in_=o[0:1, :], func=AF.Ln)
            nc.scalar.activation(out=rzb[:], in_=lnz[:], func=AF.Exp,
                                 scale=-1.0)
            nc.vector.tensor_copy(out=oraw[:], in_=o[:])  # frees psum slot
            pend_z2.append((h, p, rzb, oraw))

        def emit_z2():
            """z part 2 (deferred so the PE replicate never stalls the PE
            queue on the ACT rows): K=1 ones-matmul replicate of 1/Z, the
            normalize multiply (row 0 = Z/Z, harmless), and the SBUF->SBUF
            DMA partition shift into the pair-stacked oTn2 layout."""
            while pend_z2:
                h, p, rzb, oraw = pend_z2.pop(0)
                stage = zw.tile([HD + 1, NT], BF16, name=f"stage{h}",
                                tag="stage")
                zl = big.tile([P, NT], F32, tag="big", name=f"zl{h}")
                for ic in range(IC):
                    sl = slice(ic * 512, ic * 512 + 512)
                    nc.tensor.matmul(zl[: HD + 1, sl],
                                     lhsT=ones_rb[:, : HD + 1],
                                     rhs=rzb[:, sl], start=True, stop=True)
                nc.vector.tensor_tensor(out=stage[:], in0=oraw[:],
                                        in1=zl[: HD + 1, :], op=OP.mult)
                # even head -> rows 0-63, odd head -> 64-127
                drow = HD * (h % 2)
                nc.sync.dma_start(oTn2_sb[drow : drow + HD, p, :],
                                  stage[1 : HD + 1, :])

        def emit_scores(p, h, jt, strip):
            drow = HD * (h % 2)
            s_ps = big.tile([P, NT], F32, tag="big", name=f"s{h}_{jt}")
            for ic in range(IC):
                sl = slice(ic * 512, ic * 512 + 512)
                nc.tensor.matmul(
                    s_ps[:, sl],
                    lhsT=kT_sb[drow : drow + HD, p, jt * P : jt * P + P],
                    rhs=qT_sb[drow : drow + HD, p, sl],
                    start=True, stop=True)
            eS = es_pool.tile([P, NT], BF16, name=f"eS{h}_{jt}", tag="eS")
            nc.scalar.activation(out=eS[:], in_=s_ps[:], func=AF.Exp)
            aT = at_pool.tile([P, NT], BF16, name=f"aT{h}_{jt}", tag="aT")
            off = (28 - 4 * jt) * 32
            # offload 2 of 16 bias-multiplies per pair to the idle GPSIMD
            # (consumed 2 slots later, so its ~2.2us latency is hidden)
            on_gp = h % 2 == 0 and jt in (2, 5)
            eng = nc.gpsimd if on_gp else nc.vector
            eng.tensor_tensor(out=aT[:], in0=eS[:],
                              in1=strip[:, off : off + NT], op=OP.mult)
            at_tiles[(h, jt)] = aT

        def emit_proj(ct, pairs, out_sb, add_sb):
            csl = slice(ct * P, ct * P + P)
            yp = big.tile([P, NT], F32, tag="big", name=f"yp{ct}_{pairs[-1]}")
            for ic in range(IC):
                sl = slice(ic * 512, ic * 512 + 512)
                for i, pp in enumerate(pairs):
                    nc.tensor.matmul(yp[:, sl], lhsT=wp2_sb[:, pp, csl],
                                     rhs=oTn2_sb[:, pp, sl],
                                     start=(i == 0), stop=(pp == pairs[-1]))
            nc.vector.scalar_tensor_tensor(out=out_sb[:, ct], in0=yp[:],
                                           scalar=0.0, in1=add_sb[:, ct],
                                           op0=OP.add, op1=OP.add)

        for p in range(NPAIR):
            hA, hB = 2 * p, 2 * p + 1
            stripA = strip_tiles.pop(hA)
            stripB = strip_tiles.pop(hB)
            if p < NPAIR - 1:
                for hh in (2 * p + 2, 2 * p + 3):
                    st = strip_pool.tile([P, STRIP_W], BF16,
                                         name=f"strip{hh}", tag="strip")
                    nc.sync.dma_start(st[:], strips_d[hh])
                    strip_tiles[hh] = st

            o_tiles[hA] = ps_o.tile([HD + 1, NT], F32, tag="o", name=f"o{hA}")
            o_tiles[hB] = ps_o.tile([HD + 1, NT], F32, tag="o", name=f"o{hB}")

            for jt in range(TT):
                if jt >= 2:
                    # HAM warmers: accumulate +0 into o while PE would
                    # otherwise wait for the score psum slot (ACT-paced)
                    for ic in range(IC):
                        sl = slice(ic * 512, ic * 512 + 512)
                        nc.tensor.matmul(o_tiles[hA][:, sl],
                                         lhsT=zeros_mb[:, : HD + 1],
                                         rhs=xb_sb[:, 0, sl],
                                         start=False, stop=False,
                                         skip_group_check=True)
                for th in fillers.get((p, jt), ()):
                    th()
                emit_scores(p, hA, jt, stripA)
                emit_scores(p, hB, jt, stripB)
                # lag-2 attn@V; slots 0,1 carry the previous pair's tail
                if jt >= 2:
                    emit_attnv(hA, jt - 2)
                    emit_attnv(hB, jt - 2)
                elif p > 0:
                    emit_attnv(2 * p - 2, TT - 2 + jt)
                    emit_attnv(2 * p - 1, TT - 2 + jt)
                    if jt == 1:
                        emit_z1(2 * p - 2, p - 1)
                        emit_z1(2 * p - 1, p - 1)
                if jt == 3:
                    emit_z2()
                if p == NPAIR - 1 and jt >= 4:
                    # proj for pairs 0-2 rides in pair 3 (oTn2 p<=2 ready)
                    emit_proj(jt - 4, [0, 1, 2], y012_sb, x_sb)

        # tail: last pair's attn@V, z, proj(pair 3), output
        for jt2 in (TT - 2, TT - 1):
            emit_attnv(2 * NPAIR - 2, jt2)
            emit_attnv(2 * NPAIR - 1, jt2)
        emit_z1(2 * NPAIR - 2, NPAIR - 1)
        emit_z1(2 * NPAIR - 1, NPAIR - 1)
        emit_z2()

        for ct in range(CT):
            csl = slice(ct * P, ct * P + P)
            yp = big.tile([P, NT], F32, tag="big", name=f"yp3_{ct}")
            for ic in range(IC):
                sl = slice(ic * 512, ic * 512 + 512)
                # HAM warmer (wiped by the start=True below)
                nc.tensor.matmul(yp[:, sl], lhsT=zeros_mb[:],
                                 rhs=xb_sb[:, 0, sl], start=False,
                                 stop=False, skip_group_check=True)
                nc.tensor.matmul(yp[:, sl], lhsT=wp2_sb[:, 3, csl],
                                 rhs=oTn2_sb[:, 3, sl],
                                 start=True, stop=True)
            yw = work.tile([P, NT], F32, tag="yw", name=f"yw{ct}")
            nc.vector.scalar_tensor_tensor(out=yw[:], in0=yp[:], scalar=0.0,
                                           in1=y012_sb[:, ct],
                                           op0=OP.add, op1=OP.add)
            nc.sync.dma_start(y_d[csl, :], yw[:])

    return nc


def _legalize_waits(nc, max_waits: int = 1):
    """Split multi-wait instructions into preceding same-engine NoOps."""
    import orjson

    data = orjson.loads(mybir.module_to_json_bytes(nc.m))
    ctr = [0]

    def fix_block(block):
        out = []
        for inst in block.get("instructions", []):
            si = inst.get("sync_info") or {}
            waits = si.get("on_wait") or []
            if len(waits) > max_waits:
                for w in waits[max_waits:]:
                    ctr[0] += 1
                    nop = {
                        "name": f"I-WS{ctr[0]}",
                        "opcode": "NoOp",
                        "engine": inst["engine"],
                        "ins": [],
                        "outs": [],
                        "sync_info": {"on_wait": [w], "on_update": []},
                    }
                    if "debug" in inst:
                        nop["debug"] = inst["debug"]
                    out.append(nop)
                si = dict(si)
                si["on_wait"] = waits[:max_waits]
                inst["sync_info"] = si
            out.append(inst)
        block["instructions"] = out
        for b in block.get("blocks", []):
            fix_block(b)

    for fn in data["functions"]:
        for b in fn.get("blocks", []):
            fix_block(b)
    nc.m = mybir.module_from_json_bytes(orjson.dumps(data))
    return nc


_NC = None

BF = ml_dtypes.bfloat16


def _host_prep(x, norm_w, norm_b, wq, bq, wk, bk, wv, bv, wp, bp, rel):
    scale = HD ** -0.5
    # fold LN affine + score scale into the projection weights (exact algebra)
    wq_eff = (wq * norm_w[None, :]) * scale
    bq_eff = (bq + wq @ norm_b) * scale
    wk_eff = wk * norm_w[None, :]
    bk_eff = bk + wk @ norm_b
    wv_eff = wv * norm_w[None, :]
    bv_eff = bv + wv @ norm_b

    wqT = np.ascontiguousarray(wq_eff.T).astype(BF)
    wkT = np.ascontiguousarray(wk_eff.T).astype(BF)
    wvT = np.ascontiguousarray(wv_eff.T).astype(BF)
    # wp pair-stacked: wp2[0:64, p] = head 2p rows, wp2[64:128, p] = head 2p+1
    wpP = wp.T.reshape(HEADS, HD, CH)          # [h, d, c]
    wp2 = np.concatenate(
        [wpP[0::2], wpP[1::2]], axis=1)        # [4, 128, c]
    wp2 = np.ascontiguousarray(wp2.transpose(1, 0, 2)).astype(BF)  # [128,4,c]

    bqk = np.stack([bq_eff, bk_eff]).astype(np.float32)
    bvr = bv_eff[None, :].astype(BF)
    strips = np.exp(_build_strips(np.asarray(rel, np.float32))).astype(BF)
    # aug rows for raw-x Q/K of pair 0 (d = 0..127):
    #   q_ps = wq.T@xb + negmu*colsum(wq) + invrs*bq   (then *rs on DVE)
    # use the bf16-rounded weights for the colsums so the mu-correction
    # cancels the raw matmul's mean term exactly
    wsum_q = wqT.astype(np.float32).sum(axis=0)[:P]
    wsum_k = wkT.astype(np.float32).sum(axis=0)[:P]
    aug = np.stack([
        np.stack([wsum_q, bq_eff[:P]]),
        np.stack([wsum_k, bk_eff[:P]]),
    ], axis=1).astype(BF)   # [2 rows, 2 (q/k), 128]

    shared = {
        "wqT": wqT, "wkT": wkT, "wvT": wvT, "wp2": wp2,
        "bqk": bqk, "bv": bvr, "strips": strips, "aug": aug,
    }
    in_maps = []
    for b in range(B):
        m = dict(shared)
        xf = np.ascontiguousarray(x[b].reshape(CH, NT)).astype(np.float32)
        m["x"] = xf + bp.astype(np.float32)[:, None]   # residual with bp folded
        m["xb"] = xf.astype(BF)
        in_maps.append(m)
    return in_maps


def kernel(**inputs):
    global _NC
    if _NC is None:
        _NC = _legalize_waits(_build_nc())
    in_maps = _host_prep(**{k: np.asarray(v) for k, v in inputs.items()})
    res = run_bass_kernel_spmd(_NC, in_maps, list(range(B)))
    out = np.stack([res.results[b]["y"].reshape(CH, H, W) for b in range(B)])
    return out.astype(np.float32)


if __name__ == "__main__":
    nc = _build_nc()
    print("built OK")


# revision 43
# speedup vs baseline: 1.1734x; 1.1734x over previous
"""Trainium2 Bass kernel for nn_MHAAttention (LayerNorm2d + MHA w/ rel-pos bias + residual).

Sharding: data-parallel over batch - 8 batch elements, one per NeuronCore.

v2 design (vs the 183us baseline):
  - Head PAIRS interleaved: heads (2p, 2p+1) live at partitions 0-63 / 64-127
    of the same qT/kT tile, so their K=64 score matmuls go to different PE
    row-groups (tile_position auto) and run CONCURRENTLY (~2x score phase).
  - attn@V lag-2 carries across the pair boundary: next pair's scores are
    emitted before this pair's tail attn@V, so PE never drains (kills the
    per-head ~2.6us bubble + HAM re-throttle of the baseline).
  - v layout per head is [1 | v(64)]: the ones FIRST, so the softmax
    denominator Z lands at o partition 0 and 1/Z is computed with
    nc.vector.reciprocal_approx_fast on the (1, N) row (DVE), replicated by
    nc.gpsimd.partition_broadcast - the ScalarE ln/exp z-path is GONE;
    ScalarE does (almost) nothing but the 64 mandatory score-exp evacs.
  - Projection uses K=128 pair-stacked matmuls: wp2[0:64]=even head,
    wp2[64:128]=odd head rows; oTn2 stacked the same way via a small
    SBUF->SBUF DMA partition shift. Halves proj matmul count.
  - bp is folded into the residual x host-side; LN affine + 1/sqrt(d) are
    folded into wq/wk/wv host-side (as baseline).
  - QK for pair p+1 and the V projections are emitted INSIDE the attention
    stream (attention is ACT-bound; PE/DVE have slack), proj for pairs 0-2
    rides inside pair 3.
  - Evacuations balanced across engines: q on ScalarE (Identity + per-
    partition bias AP), k on DVE, v on ScalarE (Copy), all during attention.
"""

import sys

for _p in ("/opt/trn_rl_repo",):
    if _p not in sys.path:
        sys.path.insert(0, _p)

from contextlib import ExitStack

import numpy as np
import ml_dtypes

import concourse.bass as bass
import concourse.mybir as mybir
import concourse.tile as tile
from concourse import library_config
from concourse.bass_utils import run_bass_kernel_spmd

F32 = mybir.dt.float32
BF16 = mybir.dt.bfloat16
AF = mybir.ActivationFunctionType
OP = mybir.AluOpType

B = 8
CH = 512
H = W = 32
NT = H * W          # 1024 tokens
HEADS = 8
HD = 64
EPS = 1e-6
P = 128
CT = CH // P        # 4 channel tiles
TT = NT // P        # 8 token tiles
IC = NT // 512      # 2 free-dim chunks of 512
NPAIR = HEADS // 2  # 4 head pairs
STRIP_W = 60 * 32   # 1920
VW = 66             # per-head v stride: [1 | v(64) | pad]


def _build_strips(rel: np.ndarray) -> np.ndarray:
    """(3969, 8) rel table -> (8, 128, 1920) bias strips (see baseline docstring).

    strip[h, 32*jh_l + jw, 32*g + iw] = T_h[g - jh_l + 3, iw - jw + 31]
    bias.T block for key-tile jt is strip[:, (28-4*jt)*32 : +1024].
    """
    T = rel.reshape(63, 63, HEADS)
    jh_l = np.arange(4)[:, None, None, None]
    jw = np.arange(32)[None, :, None, None]
    g = np.arange(60)[None, None, :, None]
    iw = np.arange(32)[None, None, None, :]
    a = g - jh_l + 3
    b = iw - jw + 31
    a_b, b_b = np.broadcast_arrays(a, b)
    out = T[a_b, b_b, :]
    out = np.ascontiguousarray(np.moveaxis(out, -1, 0)).reshape(HEADS, 128, STRIP_W)
    return out


def _build_nc() -> bass.Bass:
    nc = bass.Bass()

    x_d = nc.declare_dram_parameter("x", [CH, NT], F32, isOutput=False)  # x + bp
    xb_d = nc.declare_dram_parameter("xb", [CH, NT], BF16, isOutput=False)
    wqT_d = nc.declare_dram_parameter("wqT", [CH, CH], BF16, isOutput=False)
    wkT_d = nc.declare_dram_parameter("wkT", [CH, CH], BF16, isOutput=False)
    wvT_d = nc.declare_dram_parameter("wvT", [CH, CH], BF16, isOutput=False)
    wp2_d = nc.declare_dram_parameter("wp2", [P, NPAIR, CH], BF16, isOutput=False)
    # aug rows for the raw-x Q/K of pair 0: [q/k][wsum; bias][d in dtl0]
    aug_d = nc.declare_dram_parameter("aug", [2, 2, P], BF16, isOutput=False)
    bqk_d = nc.declare_dram_parameter("bqk", [2, CH], F32, isOutput=False)
    bv_d = nc.declare_dram_parameter("bv", [1, CH], BF16, isOutput=False)
    strips_d = nc.declare_dram_parameter("strips", [HEADS, P, STRIP_W], BF16,
                                         isOutput=False)
    y_d = nc.declare_dram_parameter("y", [CH, NT], F32, isOutput=True)

    with tile.TileContext(nc) as tc, ExitStack() as ctx:
        singles = ctx.enter_context(tc.tile_pool(name="singles", bufs=1))
        work = ctx.enter_context(tc.tile_pool(name="work", bufs=4))
        es_pool = ctx.enter_context(tc.tile_pool(name="es_pool", bufs=4))
        at_pool = ctx.enter_context(tc.tile_pool(name="at_pool", bufs=6))
        strip_pool = ctx.enter_context(tc.tile_pool(name="strip_pool", bufs=4))
        zw = ctx.enter_context(tc.tile_pool(name="zw", bufs=2))
        # PSUM: big 2x(128,1024)=4 banks + o 2x(65,1024)=4 banks = 8 banks.
        big = ctx.enter_context(tc.tile_pool(name="big", bufs=2, space="PSUM"))
        ps_o = ctx.enter_context(tc.tile_pool(name="ps_o", bufs=2, space="PSUM"))

        # ---------- persistent SBUF ----------
        x_sb = singles.tile([P, CT, NT], F32)        # residual x + bp
        xb_sb = singles.tile([P, CT, NT], BF16)
        xn_sb = singles.tile([P, CT, NT], BF16)
        qT_sb = singles.tile([P, CT, NT], BF16)      # (d part, t free), pair p at ct=p
        kT_sb = singles.tile([P, CT, NT], BF16)
        v_sb = singles.tile([P, TT, HEADS * VW], BF16)
        oTn2_sb = singles.tile([P, NPAIR, NT], BF16)  # pair-stacked normalized oT
        y012_sb = singles.tile([P, CT, NT], F32)     # x + proj(pairs 0-2)

        wq_sb = singles.tile([P, CT, CH], BF16)
        wk_sb = singles.tile([P, CT, CH], BF16)
        wv_sb = singles.tile([P, CT, CH], BF16)
        wp2_sb = singles.tile([P, NPAIR, CH], BF16)
        bqk_sb = singles.tile([P, 2, CT], F32)
        bv_sb = singles.tile([1, CH], BF16)
        aug_sb = singles.tile([2, 2, P], BF16)
        augrow = singles.tile([2, NT], BF16)   # [negmu; invrs] rows
        ones_mb = singles.tile([P, P], BF16)
        ones_rb = singles.tile([1, 512], BF16)
        zeros_mb = singles.tile([P, P], BF16)  # HAM-warmer lhsT

        mu_b = singles.tile([P, NT], BF16)
        rs_b = singles.tile([P, NT], BF16)
        m2_f = singles.tile([P, NT], F32)
        ve_f = singles.tile([P, NT], F32)

        nc.vector.memset(ones_mb[:], 1.0)
        nc.vector.memset(ones_rb[:], 1.0)
        nc.vector.memset(zeros_mb[:], 0.0)

        xb_r = xb_d.rearrange("(ct p) t -> p ct t", p=P)
        # ct0 in two halves so LN stats start ASAP
        nc.sync.dma_start(xb_sb[:, 0, 0:512], xb_r[:, 0, 0:512])
        nc.sync.dma_start(xb_sb[:, 0, 512:1024], xb_r[:, 0, 512:1024])
        for ct in range(1, CT):
            nc.sync.dma_start(xb_sb[:, ct], xb_r[:, ct])
        nc.sync.dma_start(bqk_sb[:], bqk_d.rearrange("i (o p) -> p i o", p=P))
        nc.sync.dma_start(aug_sb[:], aug_d[:])
        nc.sync.dma_start(wq_sb[:], wqT_d.rearrange("(ck p) d -> p ck d", p=P))
        nc.sync.dma_start(wk_sb[:], wkT_d.rearrange("(ck p) d -> p ck d", p=P))
        nc.sync.dma_start(wv_sb[:], wvT_d.rearrange("(ck p) d -> p ck d", p=P))
        nc.sync.dma_start(bv_sb[:], bv_d[:])

        # ones column of v (first col of each head's VW block)
        v_view = v_sb[:].rearrange("p tt (h w) -> p tt h w", w=VW)
        nc.vector.memset(v_view[:, :, :, 0:1], 1.0)

        # ---------- LayerNorm stats ----------
        sum_ps = big.tile([P, NT], F32, tag="big", name="sum_ps")
        sq_ps = big.tile([P, NT], F32, tag="big", name="sq_ps")
        for ct in range(CT):
            x2 = work.tile([P, NT], BF16, name=f"x2_{ct}", tag="x2")
            nc.scalar.activation(out=x2[:], in_=xb_sb[:, ct], func=AF.Square)
            for ic in range(IC):
                sl = slice(ic * 512, ic * 512 + 512)
                nc.tensor.matmul(sum_ps[:, sl], lhsT=ones_mb[:],
                                 rhs=xb_sb[:, ct, sl],
                                 start=(ct == 0), stop=(ct == CT - 1))
                nc.tensor.matmul(sq_ps[:, sl], lhsT=ones_mb[:], rhs=x2[:, sl],
                                 start=(ct == 0), stop=(ct == CT - 1))

        # late-ish DMAs (after the critical first wave)
        x_r = x_d.rearrange("(ct p) t -> p ct t", p=P)
        for ct in range(CT):
            nc.sync.dma_start(x_sb[:, ct], x_r[:, ct])
        strip_tiles = {}
        for h in (0, 1):
            st = strip_pool.tile([P, STRIP_W], BF16, name=f"strip{h}", tag="strip")
            nc.sync.dma_start(st[:], strips_d[h])
            strip_tiles[h] = st
        nc.sync.dma_start(wp2_sb[:], wp2_d[:])

        # ---------- LN scalar chain ----------
        nc.scalar.activation(out=mu_b[:], in_=sum_ps[:], func=AF.Copy,
                             scale=1.0 / CH)
        nc.vector.tensor_tensor(out=m2_f[:], in0=mu_b[:], in1=sum_ps[:],
                                op=OP.mult)
        nc.vector.scalar_tensor_tensor(out=ve_f[:], in0=sq_ps[:],
                                       scalar=float(CH * EPS), in1=m2_f[:],
                                       op0=OP.add, op1=OP.subtract)
        nc.scalar.activation(out=ve_f[:], in_=ve_f[:], func=AF.Ln,
                             scale=1.0 / CH)
        nc.scalar.activation(out=rs_b[:], in_=ve_f[:], func=AF.Exp,
                             scale=-0.5)
        # aug rows for raw-x Q/K: negmu = -sum/CH, invrs = sqrt(var+eps)
        # (engines can only write at partition 0; DMA shifts row 1 into place)
        nc.scalar.activation(out=augrow[0:1, :], in_=sum_ps[0:1, :],
                             func=AF.Copy, scale=-1.0 / CH)
        invrs_tmp = work.tile([1, NT], BF16, name="invrs_tmp", tag="invrs")
        nc.scalar.activation(out=invrs_tmp[:], in_=ve_f[0:1, :],
                             func=AF.Exp, scale=0.5)
        nc.sync.dma_start(augrow[1:2, :], invrs_tmp[:])

        # ---------- QK / V tile emitters ----------
        def apply_ln(ct):
            nc.vector.tensor_tensor(out=xn_sb[:, ct], in0=xb_sb[:, ct],
                                    in1=mu_b[:], op=OP.subtract)
            nc.vector.tensor_tensor(out=xn_sb[:, ct], in0=xn_sb[:, ct],
                                    in1=rs_b[:], op=OP.mult)

        def emit_q1(dtl, wsb, qk, dst):
            """one projection tile (q or k) for head-pair dtl, xn-based"""
            dsl = slice(dtl * P, dtl * P + P)
            ps = big.tile([P, NT], F32, tag="big", name=f"{qk}_ps{dtl}")
            for ck in range(CT):
                for ic in range(IC):
                    sl = slice(ic * 512, ic * 512 + 512)
                    nc.tensor.matmul(ps[:, sl], lhsT=wsb[:, ck, dsl],
                                     rhs=xn_sb[:, ck, sl],
                                     start=(ck == 0), stop=(ck == CT - 1))
            nc.vector.tensor_scalar_add(out=dst[:, dtl], in0=ps[:],
                                        scalar1=bqk_sb[:, qk, dtl : dtl + 1])

        def emit_q(dtl):
            emit_q1(dtl, wq_sb, 0, qT_sb)

        def emit_k(dtl):
            emit_q1(dtl, wk_sb, 1, kT_sb)

        def emit_qk0_raw():
            """Q,K for pair 0 from RAW xb: no LN-chain dependency for the
            matmuls.  q = rs*(wq.T@xb - mu*colsum(wq) + bq*invrs), where the
            last two terms ride a K=2 aug matmul whose rhs rows [negmu;
            invrs] come from the scalar chain (only the 9th matmul waits)."""
            q_ps = big.tile([P, NT], F32, tag="big", name="q_ps0")
            k_ps = big.tile([P, NT], F32, tag="big", name="k_ps0")
            dsl = slice(0, P)
            for ck in range(CT):
                for ic in range(IC):
                    sl = slice(ic * 512, ic * 512 + 512)
                    nc.tensor.matmul(q_ps[:, sl], lhsT=wq_sb[:, ck, dsl],
                                     rhs=xb_sb[:, ck, sl],
                                     start=(ck == 0), stop=False)
                for ic in range(IC):
                    sl = slice(ic * 512, ic * 512 + 512)
                    nc.tensor.matmul(k_ps[:, sl], lhsT=wk_sb[:, ck, dsl],
                                     rhs=xb_sb[:, ck, sl],
                                     start=(ck == 0), stop=False)
            for ic in range(IC):
                sl = slice(ic * 512, ic * 512 + 512)
                nc.tensor.matmul(q_ps[:, sl], lhsT=aug_sb[:, 0, :],
                                 rhs=augrow[:, sl], start=False, stop=True)
                nc.tensor.matmul(k_ps[:, sl], lhsT=aug_sb[:, 1, :],
                                 rhs=augrow[:, sl], start=False, stop=True)
            nc.vector.tensor_tensor(out=qT_sb[:, 0], in0=q_ps[:],
                                    in1=rs_b[:], op=OP.mult)
            nc.vector.tensor_tensor(out=kT_sb[:, 0], in0=k_ps[:],
                                    in1=rs_b[:], op=OP.mult)

        def emit_v(tt2):
            """v for token tiles (2*tt2, 2*tt2+1) in one (128,1024) psum."""
            v_ps = big.tile([P, NT], F32, tag="big", name=f"v_ps{tt2}")
            for j in range(2):
                tt = 2 * tt2 + j
                tsl = slice(tt * P, tt * P + P)
                osl = slice(j * 512, j * 512 + 512)
                for ck in range(CT):
                    nc.tensor.matmul(v_ps[:, osl], lhsT=xn_sb[:, ck, tsl],
                                     rhs=wv_sb[:, ck, :],
                                     start=(ck == 0), stop=False)
                nc.tensor.matmul(v_ps[:, osl], lhsT=ones_rb[:, :P],
                                 rhs=bv_sb[:], start=False, stop=True)
            nc.vector.tensor_copy(
                out=v_view[:, 2 * tt2 : 2 * tt2 + 2, :, 1 : HD + 1],
                in_=v_ps[:].rearrange("p (tt h w) -> p tt h w", tt=2, w=HD))

        emit_qk0_raw()
        for ct in range(CT):
            apply_ln(ct)
        emit_v(0)

        # ---------- attention: pair-interleaved with lag-2 across pairs ----
        # filler schedule: slot (p, jt) -> list of thunks
        fillers = {}
        fillers[(0, 0)] = [lambda: emit_v(1)]
        fillers[(0, 2)] = [lambda: emit_v(2)]
        fillers[(0, 4)] = [lambda: emit_v(3)]
        fillers[(0, 5)] = [lambda: emit_q(1)]
        fillers[(0, 6)] = [lambda: emit_k(1)]
        fillers[(1, 2)] = [lambda: emit_q(2)]
        fillers[(1, 4)] = [lambda: emit_k(2)]
        fillers[(2, 2)] = [lambda: emit_q(3)]
        fillers[(2, 4)] = [lambda: emit_k(3)]

        o_tiles = {}       # head -> psum tile
        at_tiles = {}      # (head, jt) -> aT tile
        pend_av = []       # deferred attn@V work: (head, jt)

        def emit_attnv(h, l):
            o = o_tiles[h]
            aT = at_tiles.pop((h, l))
            for ic in range(IC):
                sl = slice(ic * 512, ic * 512 + 512)
                nc.tensor.matmul(
                    o[:, sl],
                    lhsT=v_sb[:, l, h * VW : h * VW + HD + 1],
                    rhs=aT[:, sl],
                    start=(l == 0), stop=(l == TT - 1))

        pend_z2 = []

        def emit_z1(h, p):
            """z part 1: 1/Z rows on ACT + raw-o evacuation (frees psum).

            o row 0 is Z (ones col is FIRST in v).  1/Z = exp(-ln Z) stays on
            the natural_log_exp ACT table set.
            """
            o = o_tiles.pop(h)
            lnz = zw.tile([1, NT], F32, name=f"lnz{h}", tag="lnz")
            rzb = zw.tile([1, NT], BF16, name=f"rzb{h}", tag="rzb")
            oraw = zw.tile([HD + 1, NT], BF16, name=f"oraw{h}", tag="oraw")
            nc.scalar.activation(out=lnz[:], in_=o[0:1, :], func=AF.Ln)
            nc.scalar.activation(out=rzb[:], in_=lnz[:], func=AF.Exp,
                                 scale=-1.0)
            nc.vector.tensor_copy(out=oraw[:], in_=o[:])  # frees psum slot
            pend_z2.append((h, p, rzb, oraw))

        def emit_z2():
            """z part 2 (deferred so the PE replicate never stalls the PE
            queue on the ACT rows): K=1 ones-matmul replicate of 1/Z, the
            normalize multiply (row 0 = Z/Z, harmless), and the SBUF->SBUF
            DMA partition shift into the pair-stacked oTn2 layout."""
            while pend_z2:
                h, p, rzb, oraw = pend_z2.pop(0)
                stage = zw.tile([HD + 1, NT], BF16, name=f"stage{h}",
                                tag="stage")
                zl = big.tile([P, NT], F32, tag="big", name=f"zl{h}")
                for ic in range(IC):
                    sl = slice(ic * 512, ic * 512 + 512)
                    nc.tensor.matmul(zl[: HD + 1, sl],
                                     lhsT=ones_rb[:, : HD + 1],
                                     rhs=rzb[:, sl], start=True, stop=True)
                nc.vector.tensor_tensor(out=stage[:], in0=oraw[:],
                                        in1=zl[: HD + 1, :], op=OP.mult)
                # even head -> rows 0-63, odd head -> 64-127
                drow = HD * (h % 2)
                nc.sync.dma_start(oTn2_sb[drow : drow + HD, p, :],
                                  stage[1 : HD + 1, :])

        def emit_scores(p, h, jt, strip):
            drow = HD * (h % 2)
            s_ps = big.tile([P, NT], F32, tag="big", name=f"s{h}_{jt}")
            for ic in range(IC):
                sl = slice(ic * 512, ic * 512 + 512)
                nc.tensor.matmul(
                    s_ps[:, sl],
                    lhsT=kT_sb[drow : drow + HD, p, jt * P : jt * P + P],
                    rhs=qT_sb[drow : drow + HD, p, sl],
                    start=True, stop=True)
            eS = es_pool.tile([P, NT], BF16, name=f"eS{h}_{jt}", tag="eS")
            nc.scalar.activation(out=eS[:], in_=s_ps[:], func=AF.Exp)
            aT = at_pool.tile([P, NT], BF16, name=f"aT{h}_{jt}", tag="aT")
            off = (28 - 4 * jt) * 32
            # offload 2 of 16 bias-multiplies per pair to the idle GPSIMD
            # (consumed 2 slots later, so its ~2.2us latency is hidden)
            on_gp = h % 2 == 0 and jt in (2, 5)
            eng = nc.gpsimd if on_gp else nc.vector
            eng.tensor_tensor(out=aT[:], in0=eS[:],
                              in1=strip[:, off : off + NT], op=OP.mult)
            at_tiles[(h, jt)] = aT

        def emit_proj(ct, pairs, out_sb, add_sb):
            csl = slice(ct * P, ct * P + P)
            yp = big.tile([P, NT], F32, tag="big", name=f"yp{ct}_{pairs[-1]}")
            for ic in range(IC):
                sl = slice(ic * 512, ic * 512 + 512)
                for i, pp in enumerate(pairs):
                    nc.tensor.matmul(yp[:, sl], lhsT=wp2_sb[:, pp, csl],
                                     rhs=oTn2_sb[:, pp, sl],
                                     start=(i == 0), stop=(pp == pairs[-1]))
            nc.vector.scalar_tensor_tensor(out=out_sb[:, ct], in0=yp[:],
                                           scalar=0.0, in1=add_sb[:, ct],
                                           op0=OP.add, op1=OP.add)

        for p in range(NPAIR):
            hA, hB = 2 * p, 2 * p + 1
            stripA = strip_tiles.pop(hA)
            stripB = strip_tiles.pop(hB)
            if p < NPAIR - 1:
                for hh in (2 * p + 2, 2 * p + 3):
                    st = strip_pool.tile([P, STRIP_W], BF16,
                                         name=f"strip{hh}", tag="strip")
                    nc.sync.dma_start(st[:], strips_d[hh])
                    strip_tiles[hh] = st

            o_tiles[hA] = ps_o.tile([HD + 1, NT], F32, tag="o", name=f"o{hA}")
            o_tiles[hB] = ps_o.tile([HD + 1, NT], F32, tag="o", name=f"o{hB}")

            for jt in range(TT):
                if jt >= 3:
                    # HAM warmers: accumulate +0 into o while PE would
                    # otherwise wait for the score psum slot (ACT-paced)
                    for ic in range(IC):
                        sl = slice(ic * 512, ic * 512 + 512)
                        nc.tensor.matmul(o_tiles[hA][:, sl],
                                         lhsT=zeros_mb[:, : HD + 1],
                                         rhs=xb_sb[:, 0, sl],
                                         start=False, stop=False,
                                         skip_group_check=True)
                for th in fillers.get((p, jt), ()):
                    th()
                emit_scores(p, hA, jt, stripA)
                emit_scores(p, hB, jt, stripB)
                # lag-2 attn@V; slots 0,1 carry the previous pair's tail
                if jt >= 2:
                    emit_attnv(hA, jt - 2)
                    emit_attnv(hB, jt - 2)
                elif p > 0:
                    emit_attnv(2 * p - 2, TT - 2 + jt)
                    emit_attnv(2 * p - 1, TT - 2 + jt)
                    if jt == 1:
                        emit_z1(2 * p - 2, p - 1)
                        emit_z1(2 * p - 1, p - 1)
                if jt == 3:
                    emit_z2()
                if p == NPAIR - 1 and jt >= 4:
                    # proj for pairs 0-2 rides in pair 3 (oTn2 p<=2 ready)
                    emit_proj(jt - 4, [0, 1, 2], y012_sb, x_sb)

        # tail: last pair's attn@V, z, proj(pair 3), output
        for jt2 in (TT - 2, TT - 1):
            emit_attnv(2 * NPAIR - 2, jt2)
            emit_attnv(2 * NPAIR - 1, jt2)
        emit_z1(2 * NPAIR - 2, NPAIR - 1)
        emit_z1(2 * NPAIR - 1, NPAIR - 1)
        emit_z2()

        for ct in range(CT):
            csl = slice(ct * P, ct * P + P)
            yp = big.tile([P, NT], F32, tag="big", name=f"yp3_{ct}")
            for ic in range(IC):
                sl = slice(ic * 512, ic * 512 + 512)
                nc.tensor.matmul(yp[:, sl], lhsT=wp2_sb[:, 3, csl],
                                 rhs=oTn2_sb[:, 3, sl],
                                 start=True, stop=True)
            yw = work.tile([P, NT], F32, tag="yw", name=f"yw{ct}")
            nc.vector.scalar_tensor_tensor(out=yw[:], in0=yp[:], scalar=0.0,
                                           in1=y012_sb[:, ct],
                                           op0=OP.add, op1=OP.add)
            nc.sync.dma_start(y_d[csl, :], yw[:])

    return nc


def _legalize_waits(nc, max_waits: int = 1):
    """Split multi-wait instructions into preceding same-engine NoOps."""
    import orjson

    data = orjson.loads(mybir.module_to_json_bytes(nc.m))
    ctr = [0]

    def fix_block(block):
        out = []
        for inst in block.get("instructions", []):
            si = inst.get("sync_info") or {}
            waits = si.get("on_wait") or []
            if len(waits) > max_waits:
                for w in waits[max_waits:]:
                    ctr[0] += 1
                    nop = {
                        "name": f"I-WS{ctr[0]}",
                        "opcode": "NoOp",
                        "engine": inst["engine"],
                        "ins": [],
                        "outs": [],
                        "sync_info": {"on_wait": [w], "on_update": []},
                    }
                    if "debug" in inst:
                        nop["debug"] = inst["debug"]
                    out.append(nop)
                si = dict(si)
                si["on_wait"] = waits[:max_waits]
                inst["sync_info"] = si
            out.append(inst)
        block["instructions"] = out
        for b in block.get("blocks", []):
            fix_block(b)

    for fn in data["functions"]:
        for b in fn.get("blocks", []):
            fix_block(b)
    nc.m = mybir.module_from_json_bytes(orjson.dumps(data))
    return nc


_NC = None

BF = ml_dtypes.bfloat16


def _host_prep(x, norm_w, norm_b, wq, bq, wk, bk, wv, bv, wp, bp, rel):
    scale = HD ** -0.5
    # fold LN affine + score scale into the projection weights (exact algebra)
    wq_eff = (wq * norm_w[None, :]) * scale
    bq_eff = (bq + wq @ norm_b) * scale
    wk_eff = wk * norm_w[None, :]
    bk_eff = bk + wk @ norm_b
    wv_eff = wv * norm_w[None, :]
    bv_eff = bv + wv @ norm_b

    wqT = np.ascontiguousarray(wq_eff.T).astype(BF)
    wkT = np.ascontiguousarray(wk_eff.T).astype(BF)
    wvT = np.ascontiguousarray(wv_eff.T).astype(BF)
    # wp pair-stacked: wp2[0:64, p] = head 2p rows, wp2[64:128, p] = head 2p+1
    wpP = wp.T.reshape(HEADS, HD, CH)          # [h, d, c]
    wp2 = np.concatenate(
        [wpP[0::2], wpP[1::2]], axis=1)        # [4, 128, c]
    wp2 = np.ascontiguousarray(wp2.transpose(1, 0, 2)).astype(BF)  # [128,4,c]

    bqk = np.stack([bq_eff, bk_eff]).astype(np.float32)
    bvr = bv_eff[None, :].astype(BF)
    strips = np.exp(_build_strips(np.asarray(rel, np.float32))).astype(BF)
    # aug rows for raw-x Q/K of pair 0 (d = 0..127):
    #   q_ps = wq.T@xb + negmu*colsum(wq) + invrs*bq   (then *rs on DVE)
    # use the bf16-rounded weights for the colsums so the mu-correction
    # cancels the raw matmul's mean term exactly
    wsum_q = wqT.astype(np.float32).sum(axis=0)[:P]
    wsum_k = wkT.astype(np.float32).sum(axis=0)[:P]
    aug = np.stack([
        np.stack([wsum_q, bq_eff[:P]]),
        np.stack([wsum_k, bk_eff[:P]]),
    ], axis=1).astype(BF)   # [2 rows, 2 (q/k), 128]

    shared = {
        "wqT": wqT, "wkT": wkT, "wvT": wvT, "wp2": wp2,
        "bqk": bqk, "bv": bvr, "strips": strips, "aug": aug,
    }
    in_maps = []
    for b in range(B):
        m = dict(shared)
        xf = np.ascontiguousarray(x[b].reshape(CH, NT)).astype(np.float32)
        m["x"] = xf + bp.astype(np.float32)[:, None]   # residual with bp folded
        m["xb"] = xf.astype(BF)
        in_maps.append(m)
    return in_maps


def kernel(**inputs):
    global _NC
    if _NC is None:
        _NC = _legalize_waits(_build_nc())
    in_maps = _host_prep(**{k: np.asarray(v) for k, v in inputs.items()})
    res = run_bass_kernel_spmd(_NC, in_maps, list(range(B)))
    out = np.stack([res.results[b]["y"].reshape(CH, H, W) for b in range(B)])
    return out.astype(np.float32)


if __name__ == "__main__":
    nc = _build_nc()
    print("built OK")
